# revision 40
# baseline (speedup 1.0000x reference)
"""MoE layer (top-2 of 8 experts, D=1024, F=4096) on 8 TRN2 NeuronCores.

Strategy: shard the FFN along the hidden (d_ff) axis instead of the expert
axis. Each core holds a 512-wide F-slice of ALL 8 experts' W1/W2 (16.8 MB
bf16, resident in SBUF for the whole kernel) and processes ALL routed
token-expert pairs (gathered + sorted by expert on the host). This gives
every core exactly the same, perfectly balanced workload -- sum(n_e) ~= 8192
pairs -- instead of expert-parallel's worst-expert capacity (1130 for this
routing), which puts the tensor engine at its bf16 roofline:

    per pair per core: mm1 4 f-tiles x 8 k  +  mm2 8 d-tiles x 4 k
                     = 64 PE rows -> 64 * 8192 cycles @2.4GHz ~= 218.5 us

Cores produce partial yT (contraction over their F-slice only, bf16); the
host sums the 8 partials in fp32, adds b2, applies the top-2 softmax probs
and scatter-adds into the full [S, B, D] output.  relu is per-element in F,
so F-slicing is exact: h[:, slice] depends only on W1[:, slice]/b1[slice].

All matmuls run in bf16 (1.0 PE cycles/row, same rate as fp32r but half the
DMA bytes and no >=256 free-dim constraint; measured end-to-end rel err
~5e-3 vs the 2e-2 gate). fp8 DoubleRow (0.5 cycles/row) was measured at
3.6-5% rel err on this data -- fails the gate -- so bf16 is the floor.

Input DMAs ride the SP (sync) HWDGE queue, output DMAs the Activation
queue, so a y store waiting on compute never head-of-line blocks an x/W
prefetch. Weight slab loads are interleaved with x chunk loads in need
order. A dummy-matmul warmup ramps the PE p-state to 2.4 GHz while the
first x chunk + W1 slab are still in flight.
"""

import numpy as np

D_MODEL = 1024
D_FF = 4096
N_EXPERTS = 8
TOP_K = 2
P = 128
FS = D_FF // 8        # 512   F-slice per core
KD = D_MODEL // P     # 8     k-tiles of mm1 (contraction over D)
KF = FS // P          # 4     k-tiles of mm2 (contraction over F-slice)
MD = D_MODEL // P     # 8     d-tiles of yT
CHUNK = 512           # max moving-dim chunk (PSUM bank = 512 fp32)
N_WARM = 37           # 256-row dummy matmuls sized to bridge the PE from
                      # t~=1.2us (post-memset) to x0 arrival (~5.9us) with no
                      # idle gap (a gap would reset the PE p-state ramp)

_CACHE: dict = {}


# ---------------------------------------------------------------- device ----


def _chunk_plan(seg_lens):
    """Global chunk list [(expert, pos, len)] over the concatenated padded
    segments.  Chunks within a segment are split EVENLY (no tiny tail
    chunks: sub-150-token chunks expose relu/copy latency the matmuls can't
    hide).  The first chunk is shortened so the PE can start sooner; the
    very last chunk is kept small to shrink the drain tail."""
    chunks = []
    pos = 0
    last_e = len(seg_lens) - 1
    for e, L in enumerate(seg_lens):
        parts = []
        rem = L
        if e == 0 and rem > CHUNK:
            parts.append(384)
            rem -= 384
        tail = None
        if e == last_e and rem > CHUNK:
            tail = 240
            rem -= 240
        # split rem into chunks that are multiples of 12 (L*PE_CYCLE is then
        # an integer ns so the cost model's per-matmul rounding is exact),
        # with a single remainder chunk carrying the leftover
        n = max(1, -(-rem // CHUNK))
        while True:
            base = rem // n // 12 * 12
            r = rem - (n - 1) * base
            if 12 <= r <= CHUNK:
                break
            n += 1
        sizes = [base] * (n - 1) + [r]
        sizes = sorted(sizes, reverse=True)
        assert sum(sizes) == rem and all(4 <= s <= CHUNK for s in sizes), (
            seg_lens, e, sizes)
        parts += sizes
        if tail:
            parts.append(tail)
        off = 0
        for take in parts:
            chunks.append((e, pos + off, take))
            off += take
        pos += L
    return chunks, pos


def _build(seg_lens):
    import concourse.mybir as mybir
    import concourse.tile as tile
    from concourse import bacc

    f32 = mybir.dt.float32
    bf16 = mybir.dt.bfloat16

    chunks, NT = _chunk_plan(seg_lens)
    n_chunks = len(chunks)

    nc = bacc.Bacc("TRN2", target_bir_lowering=False, debug=False)

    # host-pretiled layouts (see kernel() for the exact index maps):
    #   xh [P, KD*NT]       xh[p, KD*pos + kd*L + j] = x[kd*128+p, pos+j]
    #   w1s[e] [P, KF*KD*P] col mf*1024 + kd*128 + m = W1[e, kd*128+p, mf*128+m]
    #   w2s[e] [P, MD*KF*P] col md*512 + kf*128 + c  = W2[e, kf*128+p, md*128+c]
    #   b1s [P, E*KF]       b1s[p, e*KF+mf] = b1[e, mf*128+p]   (slice-local)
    xh = nc.dram_tensor("xh", [P, KD * NT], bf16, kind="ExternalInput").ap()
    w1s = nc.dram_tensor("w1s", [N_EXPERTS, P, KF * KD * P], bf16,
                         kind="ExternalInput").ap()
    w2s = nc.dram_tensor("w2s", [N_EXPERTS, P, MD * KF * P], bf16,
                         kind="ExternalInput").ap()
    b1s = nc.dram_tensor("b1s", [P, N_EXPERTS * KF], f32,
                         kind="ExternalInput").ap()
    yh = nc.dram_tensor("yh", [P, MD * NT], bf16, kind="ExternalOutput").ap()

    with tile.TileContext(nc) as tc:
        with (
            tc.tile_pool(name="const", bufs=1) as const,
            tc.tile_pool(name="wp", bufs=1) as wp,
            tc.tile_pool(name="xp", bufs=4) as xp,
            tc.tile_pool(name="hp", bufs=2) as hp,
            tc.tile_pool(name="yp", bufs=3) as yp,
            tc.tile_pool(name="ps1", bufs=3, space="PSUM") as ps1p,
            tc.tile_pool(name="ps2", bufs=4, space="PSUM") as ps2p,
        ):
            # b1 rides the Activation HWDGE queue so it lands early without
            # consuming a slot in the SP input stream
            b1_sb = const.tile([P, N_EXPERTS * KF], f32, tag="b1")
            nc.scalar.dma_start(b1_sb[:], b1s[:, :])

            # PE p-state warmup on a zeroed scratch tile (no DMA dependency):
            # the clock reaches 2.4 GHz while the first x/W1 transfers land.
            warm = const.tile([P, 256], bf16, tag="warm")
            nc.vector.memset(warm[:], 0.0)
            wps = ps1p.tile([P, 256], f32, tag="ps1")
            for i in range(N_WARM):
                nc.tensor.matmul(wps[:], warm[:, :P], warm[:],
                                 start=(i == 0), stop=(i == N_WARM - 1))
            nc.vector.tensor_copy(warm[:, :256], wps[:])

            # Input DMA emission order == SP queue service order, and the
            # queue is in-order: an x prefetch waiting on buffer rotation
            # head-of-line blocks everything behind it.  So weight slabs are
            # always queued BEFORE the (potentially blocking) x prefetch of
            # the same iteration, and the early slabs ride the preamble
            # between the first 5 (fresh-buffer, wait-free) x chunks.
            w1_sb = [None] * N_EXPERTS
            w2_sb = [None] * N_EXPERTS

            def emit_w1(e):
                w1_sb[e] = wp.tile([P, KF * KD * P], bf16,
                                   tag=f"w1_{e}", name=f"w1sb{e}")
                nc.sync.dma_start(w1_sb[e][:], w1s[e][:, :])

            def emit_w2(e):
                w2_sb[e] = wp.tile([P, MD * KF * P], bf16,
                                   tag=f"w2_{e}", name=f"w2sb{e}")
                nc.sync.dma_start(w2_sb[e][:], w2s[e][:, :])

            x_sb = [None] * n_chunks

            def emit_x(c):
                e, pos, L = chunks[c]
                x_sb[c] = xp.tile([P, KD * L], bf16, tag="x", name=f"xsb{c}")
                nc.sync.dma_start(x_sb[c][:], xh[:, KD * pos:KD * (pos + L)])

            XP = 4                            # x prefetch depth (= xp bufs)
            # first W1/W2 slabs in halves: mm1 f-tiles 0-1 need only half of
            # w1s[0], mm2 d-tiles 0-3 only half of w2s[0]
            HW1 = KF * KD * P // 2
            HW2 = MD * KF * P // 2
            w1_sb[0] = wp.tile([P, KF * KD * P], bf16, tag="w1_0",
                               name="w1sb0")
            nc.sync.dma_start(w1_sb[0][:, :HW1], w1s[0][:, :HW1])
            # first x chunk in two kd-halves: mm1's k-steps 0-3 start after
            # only half the chunk has landed
            e0, pos0, L0 = chunks[0]
            x_sb[0] = xp.tile([P, KD * L0], bf16, tag="x", name="xsb0")
            HX = KD * L0 // 2
            nc.sync.dma_start(x_sb[0][:, :HX], xh[:, KD * pos0:KD * pos0 + HX])
            nc.sync.dma_start(x_sb[0][:, HX:], xh[:, KD * pos0 + HX:
                                                   KD * (pos0 + L0)])
            nc.sync.dma_start(w1_sb[0][:, HW1:], w1s[0][:, HW1:])
            w2_sb[0] = wp.tile([P, MD * KF * P], bf16, tag="w2_0",
                               name="w2sb0")
            nc.sync.dma_start(w2_sb[0][:, :HW2], w2s[0][:, :HW2])
            nc.sync.dma_start(w2_sb[0][:, HW2:], w2s[0][:, HW2:])
            emit_w1(1)
            for c in range(1, min(XP, n_chunks)):
                emit_x(c)
                if c == 1:
                    emit_w2(1)

            # just-in-time weight drip: expert e's pair lands ~3 chunks
            # before its segment starts, so weights never crowd out the x
            # stream on the serialized DMA engine
            seg_start_chunk = {}
            for ci, (ce, _, _) in enumerate(chunks):
                seg_start_chunk.setdefault(ce, ci)
            w_at = {}
            for ew in range(2, N_EXPERTS):
                w_at.setdefault(max(0, seg_start_chunk[ew] - 3), []).append(ew)

            for c, (e, pos, L) in enumerate(chunks):
                for ew in w_at.get(c, []):
                    emit_w1(ew)
                    emit_w2(ew)
                if c + XP < n_chunks:
                    emit_x(c + XP)

                # ---- mm1: h[4 f-tiles, L] = relu(W1s.T @ x + b1s) ----
                h_tiles = []
                for mf in range(KF):
                    ps = ps1p.tile([P, L], f32, tag="ps1")
                    for kd in range(KD):
                        nc.tensor.matmul(
                            ps[:],
                            w1_sb[e][:, mf * (KD * P) + kd * P:
                                     mf * (KD * P) + (kd + 1) * P],
                            x_sb[c][:, kd * L:(kd + 1) * L],
                            start=(kd == 0), stop=(kd == KD - 1))
                    h = hp.tile([P, L], bf16, tag=f"h{mf}")
                    nc.scalar.activation(
                        h[:], ps[:],
                        mybir.ActivationFunctionType.Relu,
                        bias=b1_sb[:, e * KF + mf:e * KF + mf + 1])
                    h_tiles.append(h)

                # ---- mm2: y[8 d-tiles, L] = W2s.T @ h (partial over F) ----
                # The last k-step (kf=3) of each d-tile is deferred by TWO
                # d-tiles so the relu producing h[3] has ~9 matmuls of slack
                # instead of racing the first d-tile's accumulation.  (Any PE
                # idle gap also resets the p-state ramp to half clock for
                # 3us, so even ~100ns relu races are worth designing out.)
                y_slab = yp.tile([P, MD * L], bf16, tag="y")
                ps_md = [None] * MD
                # stores ride the Pool/SWDGE queue (keeps relu's Activation
                # queue and the SP input queue clean); the final chunk's
                # stores use the by-then-idle SP HWDGE path, which has lower
                # issue latency, to shrink the drain tail
                store_dma = (nc.sync.dma_start if c == n_chunks - 1
                             else nc.gpsimd.dma_start)

                def finish_md(md):
                    nc.tensor.matmul(
                        ps_md[md][:],
                        w2_sb[e][:, md * (KF * P) + (KF - 1) * P:
                                 (md + 1) * (KF * P)],
                        h_tiles[KF - 1][:],
                        start=False, stop=True)
                    nc.vector.tensor_copy(y_slab[:, md * L:(md + 1) * L],
                                          ps_md[md][:])
                    if c == n_chunks - 1:
                        # final chunk: small stores alternating the two HWDGE
                        # queues; the last two d-tiles store solo so the
                        # drain waits only on a single-tile transfer
                        if md in (1, 3, 5):
                            q = {1: nc.sync, 3: nc.scalar, 5: nc.sync}[md]
                            q.dma_start(
                                yh[:, MD * pos + (md - 1) * L:
                                   MD * pos + (md + 1) * L],
                                y_slab[:, (md - 1) * L:(md + 1) * L])
                        elif md == 7:
                            nc.scalar.dma_start(
                                yh[:, MD * pos + 7 * L:MD * pos + 8 * L],
                                y_slab[:, 7 * L:8 * L])
                        elif md == 6:
                            nc.sync.dma_start(
                                yh[:, MD * pos + 6 * L:MD * pos + 7 * L],
                                y_slab[:, 6 * L:7 * L])
                    elif md == MD // 2 - 1:
                        # first-half store leaves while the second half is
                        # still being produced
                        store_dma(
                            yh[:, MD * pos:MD * pos + (MD // 2) * L],
                            y_slab[:, :(MD // 2) * L])

                for md in range(MD):
                    ps_md[md] = ps2p.tile([P, L], f32, tag="ps2",
                                          name=f"ps2md{md % 4}")
                    for kf in range(KF - 1):
                        nc.tensor.matmul(
                            ps_md[md][:],
                            w2_sb[e][:, md * (KF * P) + kf * P:
                                     md * (KF * P) + (kf + 1) * P],
                            h_tiles[kf][:],
                            start=(kf == 0), stop=False)
                    if md > 1:
                        finish_md(md - 2)
                if c == n_chunks - 1:
                    # md7 finishes first so its copy+store (the drain's
                    # critical path) starts one matmul earlier
                    finish_md(MD - 1)
                    finish_md(MD - 2)
                else:
                    finish_md(MD - 2)
                    finish_md(MD - 1)
                if c != n_chunks - 1:
                    store_dma(
                        yh[:, MD * pos + (MD // 2) * L:MD * (pos + L)],
                        y_slab[:, (MD // 2) * L:])

    nc.compile()
    return nc


def _get_program(seg_lens):
    key = tuple(seg_lens)
    if key not in _CACHE:
        _CACHE[key] = _build(seg_lens)
    return _CACHE[key]


# ------------------------------------------------------------------ host ----


def kernel(x, gate_w, gate_b, w1, b1, w2, b2):
    import ml_dtypes
    from concourse import bass_utils

    bf16 = ml_dtypes.bfloat16

    S, B, D = x.shape
    N = S * B
    x = np.ascontiguousarray(np.asarray(x, dtype=np.float32))
    x_flat = x.reshape(N, D)

    # --- gate (host, fp64 for a faithful top-k) ---
    scores = x_flat.astype(np.float64) @ np.asarray(gate_w, np.float64)
    scores += np.asarray(gate_b, np.float64)
    order = np.argsort(-scores, axis=1, kind="stable")
    top_idx = order[:, :TOP_K]                       # [N, K]
    top_val = np.take_along_axis(scores, top_idx, axis=1)
    top_val -= top_val.max(axis=1, keepdims=True)
    e_val = np.exp(top_val)
    probs = (e_val / e_val.sum(axis=1, keepdims=True)).astype(np.float32)

    # --- gather pairs per expert, pad each segment to a multiple of 4 ---
    idx_e = [np.where((top_idx == e).any(axis=1))[0] for e in range(N_EXPERTS)]
    p_e = []
    for e in range(N_EXPERTS):
        sel = (top_idx[idx_e[e]] == e)
        p_e.append((probs[idx_e[e]] * sel).sum(axis=1))
    seg_lens = [max(4, -(-len(i) // 4) * 4) for i in idx_e]

    nc = _get_program(seg_lens)
    chunks, NT = _chunk_plan(seg_lens)

    # --- pack inputs ---
    xg = np.zeros((D, NT), bf16)                      # gathered, [D, NT]
    offs = np.cumsum([0] + seg_lens)
    for e in range(N_EXPERTS):
        xg[:, offs[e]:offs[e] + len(idx_e[e])] = x_flat[idx_e[e]].T
    xh = np.empty((P, KD * NT), bf16)
    for (_, pos, L) in chunks:
        xh[:, KD * pos:KD * (pos + L)] = (
            xg[:, pos:pos + L].reshape(KD, P, L)
            .swapaxes(0, 1).reshape(P, KD * L))

    w1 = np.asarray(w1, np.float32)
    b1 = np.asarray(b1, np.float32)
    w2 = np.asarray(w2, np.float32)
    b2 = np.asarray(b2, np.float32)

    in_maps = []
    for core in range(N_EXPERTS):
        sl = slice(core * FS, (core + 1) * FS)
        # w1s[e][p, mf*1024 + kd*128 + m] = W1[e, kd*128+p, core*512+mf*128+m]
        w1c = (w1[:, :, sl].astype(bf16)
               .reshape(N_EXPERTS, KD, P, KF, P)
               .transpose(0, 2, 3, 1, 4)
               .reshape(N_EXPERTS, P, KF * KD * P))
        # w2s[e][p, md*512 + kf*128 + c] = W2[e, core*512+kf*128+p, md*128+c]
        w2c = (w2[:, sl, :].astype(bf16)
               .reshape(N_EXPERTS, KF, P, MD, P)
               .transpose(0, 2, 3, 1, 4)
               .reshape(N_EXPERTS, P, MD * KF * P))
        b1c = (b1[:, sl].reshape(N_EXPERTS, KF, P)
               .transpose(2, 0, 1).reshape(P, N_EXPERTS * KF))
        in_maps.append({
            "xh": xh,
            "w1s": np.ascontiguousarray(w1c),
            "w2s": np.ascontiguousarray(w2c),
            "b1s": np.ascontiguousarray(b1c),
        })

    res = bass_utils.run_bass_kernel_spmd(
        nc, in_maps, core_ids=list(range(N_EXPERTS)))

    # --- combine partials on host ---
    ys = np.zeros((P, MD * NT), np.float32)
    for core in range(N_EXPERTS):
        ys += res.results[core]["yh"].astype(np.float32)
    yT = np.empty((D, NT), np.float32)                # [D, NT]
    for (_, pos, L) in chunks:
        yT[:, pos:pos + L] = (
            ys[:, MD * pos:MD * (pos + L)].reshape(P, MD, L)
            .swapaxes(0, 1).reshape(D, L))

    out = np.zeros((N, D), np.float32)
    for e in range(N_EXPERTS):
        cnt = len(idx_e[e])
        y_seg = yT[:, offs[e]:offs[e] + cnt].T + b2[e]
        out[idx_e[e]] += p_e[e][:, None] * y_seg      # idx_e[e] is unique
    return out.reshape(S, B, D)
